# revision 1
# baseline (speedup 1.0000x reference)
"""Trainium2 Bass kernel for nn_CAML_53240414601378.

Embedding lookup -> Conv1d(k=4, pad=2) -> tanh -> per-label attention
pooling -> logits. Data-parallel over batch across 8 NeuronCores
(4 batches per core); small params replicated.

Structure per batch (per core):
- dma_gather(transpose=True) pulls bf16 embedding rows straight into
  (e%128, e//128, s) layout -- no on-chip transpose. Batches 0 and 3
  gather in two halves so conv can start earlier / finish later against
  the serial SWDGE descriptor-generation chain.
- conv1d(k=4) = 4 shifted bf16 matmuls x 2 E-chunks x 2 F-chunks into
  PSUM; boundaries handled with shrunken-N matmuls (no padding).
- scores = U_w @ H and t = final_w @ H come from ONE matmul per
  (F-chunk, seq-tile) with a combined stationary operand (U_w -> psum
  rows 0..49, final_w -> rows 64..113).
- online softmax: per seq-tile partial (-max, Z, num) computed straight
  off the PSUM tile; tiny (50, 9) combine at the end. logits =
  sum_s alpha * t + final_b -- the (B,L,F) intermediate of the
  reference is never materialized.
"""

import numpy as np
import ml_dtypes

import concourse.bass as bass
import concourse.tile as tile
from concourse.tile import add_dep_helper
from concourse import bacc, mybir
from concourse.bass_utils import run_bass_kernel_spmd

B, S = 32, 4096
VOCAB, E, F, L = 30522, 256, 256, 50
SO = S + 1  # conv output length (4097)
N_CORES = 8
BPC = B // N_CORES  # batches per core
BF16 = mybir.dt.bfloat16
FP32 = mybir.dt.float32
NT, TN = 8, 512  # full seq tiles covering t in [0, 4096)
NJ = NT + 1      # score tiles (8x512 + 1)

_cache = {}


def _conv_mms(t0, n):
    """Conv matmul pieces for output cols [t0, t0+n): (k, lo, hi, off),
    full-width first so start=True covers the whole psum range."""
    shifts = []
    for k in range(4):
        lo = max(0, t0 + k - 2)
        hi = min(S, t0 + k - 2 + n)
        shifts.append((k, lo, hi, lo - (t0 + k - 2)))
    shifts.sort(key=lambda s: -(s[2] - s[1]))
    return shifts


def build_nc():
    nc = bacc.Bacc("TRN2", target_bir_lowering=False, debug=False,
                   num_devices=N_CORES)

    emb_ap = nc.dram_tensor("emb", (VOCAB, E), BF16, kind="ExternalInput").ap()
    idx_ap = nc.dram_tensor("idx", (128, BPC * S // 16), mybir.dt.int16,
                            kind="ExternalInput").ap()
    w_ap = nc.dram_tensor("wconv", (128, 16, 128), BF16,
                          kind="ExternalInput").ap()
    uw_ap = nc.dram_tensor("uwfw", (128, 2, 114), BF16,
                           kind="ExternalInput").ap()
    cb_ap = nc.dram_tensor("cbias", (128, 2), FP32, kind="ExternalInput").ap()
    fb_ap = nc.dram_tensor("fbias", (L, 1), FP32, kind="ExternalInput").ap()
    out_ap = nc.dram_tensor("out", (L, BPC), FP32, kind="ExternalOutput").ap()

    with tile.TileContext(nc) as tc:
        with (
            tc.tile_pool(name="const", bufs=1) as const,
            tc.tile_pool(name="xh", bufs=3) as xh,     # gather half-tiles
            tc.tile_pool(name="xp", bufs=2) as xp,     # full-batch x
            tc.tile_pool(name="hp", bufs=2) as hp,
            tc.tile_pool(name="ep", bufs=3) as ep,     # exp scratch tiles
            tc.tile_pool(name="pp", bufs=2) as pp,     # per-batch partials
            tc.tile_pool(name="small", bufs=8) as small,
            tc.tile_pool(name="psum", bufs=2, space="PSUM") as psum,
            tc.tile_pool(name="psum_st", bufs=4, space="PSUM") as psum_st,
        ):
            # ---- constants (loaded once) ----
            idx_sb = const.tile([128, BPC * S // 16], mybir.dt.int16)
            nc.gpsimd.dma_start(idx_sb[:], idx_ap[:])
            w_sb = const.tile([128, 16, 128], BF16)
            nc.sync.dma_start(w_sb[:], w_ap[:])
            uw_sb = const.tile([128, 2, 114], BF16)
            nc.sync.dma_start(uw_sb[:], uw_ap[:])
            cb_sb = const.tile([128, 2], FP32)
            nc.sync.dma_start(cb_sb[:], cb_ap[:])
            fb_sb = const.tile([L, 1], FP32)
            nc.sync.dma_start(fb_sb[:], fb_ap[:])
            out_sb = const.tile([L, BPC], FP32)

            IPB = S // 16  # idx columns per batch
            HS = S // 2

            for b in range(BPC):
                # ---- embedding gather -> (e%128, e//128, s), bf16 ----
                # two halves per batch: conv starts on half A while the
                # serial SWDGE descriptor generation works on half B
                # chunk plan: small first chunks on batch 0 (early PE
                # start), tiny last chunk on the final batch (short tail
                # after the serial descriptor-generation chain ends)
                if b == 0:
                    plan = [512, 512, 1024, 2048]
                elif b == BPC - 1:
                    plan = [1024, 1024, 1024, 896, 128]
                else:
                    plan = [1024, 1024, 1024, 1024]
                segs = []
                g0 = 0
                prev_g = None
                for q, cs in enumerate(plan):
                    xt = xh.tile([128, 2, cs], BF16, tag=f"q{q}")
                    gi = nc.gpsimd.dma_gather(
                        out_ap=xt[:], in_ap=emb_ap[:],
                        idxs_ap=idx_sb[:, b * IPB + g0 // 16:
                                       b * IPB + (g0 + cs) // 16],
                        num_idxs=cs, num_idxs_reg=cs, elem_size=E,
                        transpose=True, single_packet=False)
                    if prev_g is not None:
                        add_dep_helper(prev_g.ins, gi.ins, False,
                                       "gather chain order")
                    prev_g = gi
                    segs.append((xt, g0, g0 + cs))
                    g0 += cs

                def rhs_pieces(lo, hi):
                    out = []
                    for (xt, g0, g1) in segs:
                        a, bnd = max(lo, g0), min(hi, g1)
                        if a < bnd:
                            out.append((xt, a - g0, bnd - g0, a - lo))
                    return out

                H = hp.tile([128, 2, SO], BF16, tag="H")
                nmx = pp.tile([L, NJ], FP32, tag="nmx")  # -max per tile
                zp = pp.tile([L, NJ], FP32, tag="zp")    # partial Z
                np_ = pp.tile([L, NJ], FP32, tag="np")   # partial num

                def score_tile(j, n):
                    """Combined scores/t matmul for H cols [j*TN, +n) and
                    the online-softmax partials for that tile."""
                    t0 = j * TN
                    pst = psum_st.tile([114, TN], FP32, tag="st")
                    for fc in range(2):
                        nc.tensor.matmul(
                            pst[:, 0:n], uw_sb[:, fc, :], H[:, fc, t0:t0 + n],
                            start=(fc == 0), stop=(fc == 1),
                        )
                    nc.vector.reduce_max(nmx[:, j:j + 1], pst[0:L, 0:n],
                                         axis=mybir.AxisListType.X,
                                         negate=True)
                    e_sb = ep.tile([L, TN], FP32, tag="e")
                    nc.scalar.activation(
                        e_sb[:, 0:n], pst[0:L, 0:n],
                        mybir.ActivationFunctionType.Exp,
                        bias=nmx[:, j:j + 1], accum_out=zp[:, j:j + 1],
                    )
                    nc.vector.tensor_mul(e_sb[:, 0:n], e_sb[:, 0:n],
                                         pst[64:64 + L, 0:n])
                    nc.vector.reduce_sum(np_[:, j:j + 1], e_sb[:, 0:n],
                                         axis=mybir.AxisListType.X)

                # ---- conv1d(k=4) + bias + tanh + scores, per seq tile ----
                for j in range(NT):
                    t0 = j * TN
                    for fc in range(2):
                        ph = psum.tile([128, TN], FP32, tag=f"h{fc}")
                        mms = []
                        for (k, lo, hi, off) in _conv_mms(t0, TN):
                            for (xt, a, bnd, rel) in rhs_pieces(lo, hi):
                                for ec in range(2):
                                    mms.append((k, ec, xt, a, bnd, off + rel))
                        for i, (k, ec, xt, a, bnd, off) in enumerate(mms):
                            nc.tensor.matmul(
                                ph[:, off:off + (bnd - a)],
                                w_sb[:, k * 4 + ec * 2 + fc, :],
                                xt[:, ec, a:bnd],
                                start=(i == 0), stop=(i == len(mms) - 1),
                            )
                        nc.scalar.activation(
                            H[:, fc, t0:t0 + TN], ph[:],
                            mybir.ActivationFunctionType.Tanh,
                            bias=cb_sb[:, fc:fc + 1],
                        )
                    score_tile(j, TN)

                # last output column t = 4096 (x cols 4094..4095)
                lastsz = segs[-1][2] - segs[-1][1]
                for fc in range(2):
                    ph9 = psum.tile([128, 1], FP32, tag=f"h{fc}")
                    i = 0
                    for k in range(2):
                        for ec in range(2):
                            nc.tensor.matmul(
                                ph9[:, 0:1],
                                w_sb[:, k * 4 + ec * 2 + fc, :],
                                segs[-1][0][:, ec, lastsz - 2 + k:
                                            lastsz - 1 + k],
                                start=(i == 0), stop=(i == 3),
                            )
                            i += 1
                    nc.scalar.activation(
                        H[:, fc, S:SO], ph9[:],
                        mybir.ActivationFunctionType.Tanh,
                        bias=cb_sb[:, fc:fc + 1],
                    )
                score_tile(NT, 1)


                # ---- combine partials -> logits ----
                nm = small.tile([L, 1], FP32, tag="nm")  # -(global max)
                nc.vector.reduce_max(nm[:], nmx[:], axis=mybir.AxisListType.X,
                                     op=mybir.AluOpType.min)
                wj = small.tile([L, NJ], FP32, tag="wj")
                nc.scalar.activation(
                    wj[:], nmx[:], mybir.ActivationFunctionType.Exp,
                    bias=nm[:], scale=-1.0,
                )
                wz = small.tile([L, NJ], FP32, tag="wz")
                nc.vector.tensor_mul(wz[:], wj[:], zp[:])
                zsum = small.tile([L, 1], FP32, tag="zsum")
                nc.vector.reduce_sum(zsum[:], wz[:], axis=mybir.AxisListType.X)
                nc.vector.tensor_mul(wj[:], wj[:], np_[:])
                nsum = small.tile([L, 1], FP32, tag="nsum")
                nc.vector.reduce_sum(nsum[:], wj[:], axis=mybir.AxisListType.X)
                zr = small.tile([L, 1], FP32, tag="zr")
                nc.vector.reciprocal(zr[:], zsum[:])
                sm = small.tile([L, 1], FP32, tag="sm")
                nc.vector.tensor_mul(sm[:], nsum[:], zr[:])
                nc.vector.tensor_add(out_sb[:, b:b + 1], sm[:], fb_sb[:])

            nc.sync.dma_start(out_ap[:], out_sb[:])

    nc.compile()
    return nc


def _prep_shared(emb_table, conv_w, conv_b, U_w, final_w, final_b):
    emb_bf = np.ascontiguousarray(emb_table.astype(ml_dtypes.bfloat16))

    # wconv[e_lo, k*4 + ec*2 + fc, f_lo] = conv_w[fc*128+f, ec*128+e, k]
    W = np.empty((128, 16, 128), np.float32)
    for k in range(4):
        for ec in range(2):
            for fc in range(2):
                W[:, k * 4 + ec * 2 + fc, :] = conv_w[
                    fc * 128:(fc + 1) * 128, ec * 128:(ec + 1) * 128, k].T
    W = np.ascontiguousarray(W.astype(ml_dtypes.bfloat16))

    # uwfw[f_lo, fc, j]: j<50 -> U_w[j, fc*128+f_lo];
    # j in [64,114) -> final_w[j-64, fc*128+f_lo]; rest zero
    UW = np.zeros((128, 2, 114), np.float32)
    UW[:, :, 0:L] = U_w.T.reshape(2, 128, L).transpose(1, 0, 2)
    UW[:, :, 64:64 + L] = final_w.T.reshape(2, 128, L).transpose(1, 0, 2)
    UW = np.ascontiguousarray(UW.astype(ml_dtypes.bfloat16))

    CB = np.ascontiguousarray(conv_b.reshape(2, 128).T.astype(np.float32))
    FB = np.ascontiguousarray(final_b.reshape(L, 1).astype(np.float32))
    return emb_bf, W, UW, CB, FB


def kernel(input_ids, emb_table, conv_w, conv_b, U_w, final_w, final_b):
    import os
    ids = np.asarray(input_ids)
    emb_table = np.asarray(emb_table, dtype=np.float32)
    conv_w = np.asarray(conv_w, dtype=np.float32)
    conv_b = np.asarray(conv_b, dtype=np.float32)
    U_w = np.asarray(U_w, dtype=np.float32)
    final_w = np.asarray(final_w, dtype=np.float32)
    final_b = np.asarray(final_b, dtype=np.float32)

    if "nc" not in _cache:
        _cache["nc"] = build_nc()
    nc = _cache["nc"]

    emb_bf, W, UW, CB, FB = _prep_shared(
        emb_table, conv_w, conv_b, U_w, final_w, final_b)

    ids16 = ids.astype(np.int16)  # vocab 30522 < 2**15
    in_maps = []
    for c in range(N_CORES):
        cid = ids16[c * BPC:(c + 1) * BPC]  # (BPC, S)
        # position i -> [i % 16, i // 16], batches along axis 1; the
        # 16-row block is replicated to all 8 gpsimd cores (128 rows)
        blk = np.concatenate(
            [cid[b].reshape(S // 16, 16).T for b in range(BPC)], axis=1)
        idx = np.tile(blk, (8, 1))
        in_maps.append({
            "emb": emb_bf, "idx": np.ascontiguousarray(idx),
            "wconv": W, "uwfw": UW, "cbias": CB, "fbias": FB,
        })

    trace = bool(int(os.environ.get("KERNEL_TRACE", "0")))
    res = run_bass_kernel_spmd(nc, in_maps, core_ids=list(range(N_CORES)),
                               trace=trace)
    _cache["last_result"] = res

    out = np.concatenate(
        [res.results[c]["out"].T for c in range(N_CORES)], axis=0)
    return np.ascontiguousarray(out.astype(np.float32))

